# revision 23
# baseline (speedup 1.0000x reference)
"""EnhancedGCNII on 8 Trainium2 NeuronCores.

Strategy (row-sharded nodes, host-transposed fp8 adjacency), v2:
  - Core c owns node rows Rc = [c*1024, (c+1)*1024).
  - Host ships AT_c = adj[Rc, :].T as fp8e4 in r-half-major layout
    [128, 2, 64, 512]: partition p, (rh, kc, rin) with rh the local-node
    half, kc the global 128-node chunk, rin the node within the half.
    The rh-0 half streams first so its degrees (colsums) complete at
    half-stream, letting the first AllGather of dinv*h0 fire early.
  - AT streams as 16 half-slabs round-robin over the sync/scalar/gpsimd
    DMA queues; x + all weights ride the vector queue.
  - A tiny dummy AllGather issues first so the NEFF-level collectives
    barrier/init cost is absorbed during the AT stream.
  - Degrees: deg[r] = colsum of AT via all-ones stationary fp8 DoubleRow
    matmuls consuming half-slabs as they land; sqrt/reciprocal on tiny
    extracted tiles only.
  - Associativity: A_hat @ (H W') = (A_hat @ H) @ W', so per layer only
    ONE 128-feature SpMM: S^T = P'^T @ AT with P' = dinv*h gathered fp8.
    The self-loop term is folded into the epilogue without a PE matmul:
    AH = dinv*S + dinv^2*h, i.e. ah = st*b_d1 + hT*b_d2.
  - SpMM consumes gathered halves in arrival order with st0/st1 (output
    node halves) interleaved per input half, so all non-gated matmuls
    run before the second half-gather lands.
  - gcnii constant term G0_i = M'_i @ (a/(1-a) h0) precomputed; per
    layer gcnii = relu(AH@M' + G0 + bg), linear = AH@W' + b'.
  - psb (gathered stationary) loads split over 4 DMA queues; the
    tensor-queue piece is enqueued right where the PE would stall
    anyway.
  - Everything stays in transposed [feat, node] layout; biases are
    per-partition scalars. logits^T = fc_out_w^T @ h^T, host transposes.
"""

import sys
import types

sys.path.insert(0, "/opt/trn_rl_repo")

# ---------------------------------------------------------------------------
# Environment shims (axon container):
#  - antenv.axon_hooks is absent; register the NTFF profile hook ourselves so
#    trace=True yields exec_time_ns.
#  - no artifact bucket; skip uploads.
#  - walrus in this container allows only ONE semaphore wait on the CTRL
#    instruction Tile emits as the kernel-tail drain; split the waits across
#    sequential NOPs.
# ---------------------------------------------------------------------------
import antenv  # noqa: E402

if "antenv.axon_hooks" not in sys.modules:
    _mod = types.ModuleType("antenv.axon_hooks")
    _hook = [None]
    _mod.set_axon_ntff_profile_hook = lambda h: _hook.__setitem__(0, h)
    _mod.get_axon_ntff_profile_hook = lambda: _hook[0]
    sys.modules["antenv.axon_hooks"] = _mod
    antenv.axon_hooks = _mod
    try:
        from trn_agent_boot.trn_boot import _ntff_profile_via_ctypes

        _mod.set_axon_ntff_profile_hook(
            _ntff_profile_via_ctypes("/opt/axon/libaxon_pjrt.so")
        )
    except Exception as _e:
        print(f"ntff hook registration failed: {_e}", file=sys.stderr)

import numpy as np  # noqa: E402
import ml_dtypes  # noqa: E402
import concourse.bass as bass  # noqa: E402
import concourse.bacc as bacc  # noqa: E402
import concourse.mybir as mybir  # noqa: E402
import concourse.tile as tile  # noqa: E402
from concourse import bass_utils  # noqa: E402

bass_utils.upload_artifacts = lambda tmpdir: f"local://{tmpdir}"

_MAX_DRAIN_WAITS = 1


def _split_drain_and_barrier(self, tick_clock, wait_clock):
    nc = self.nc
    carrier = nc.sync.nop(hint="drain_wait_carrier", nofuse=True)
    wait_clock.add_sem_waits(
        carrier.ins, tile.ScopedClock({None: tick_clock.global_clock})
    )
    si = carrier.ins.sync_info
    if si is not None and len(si.on_wait) > _MAX_DRAIN_WAITS:
        waits = list(si.on_wait)
        carrier.ins.sync_info = mybir.SyncInfo(
            on_wait=waits[:_MAX_DRAIN_WAITS], on_update=list(si.on_update)
        )
        for i in range(_MAX_DRAIN_WAITS, len(waits), _MAX_DRAIN_WAITS):
            extra = nc.sync.nop(hint="drain_wait_split", nofuse=True)
            extra.ins.sync_info = mybir.SyncInfo(
                on_wait=waits[i : i + _MAX_DRAIN_WAITS], on_update=[]
            )
    nc.sync.drain()
    nc.all_engine_barrier()
    assert self.sems is not None
    popped = nc._tile_sem_poison_stack.pop()
    assert popped is self._sem_poison
    nc.clear_and_free_semaphores(list(self.sems.allocated().values()))
    nc.all_engine_barrier()


tile.TileContext._drain_and_barrier = _split_drain_and_barrier

# ---------------------------------------------------------------------------
# Problem constants (hardcoded per the harness contract)
# ---------------------------------------------------------------------------
import math  # noqa: E402

N, NFEAT, NHID, NCLASS, NLAYERS = 8192, 500, 128, 40, 4
ALPHA, GAMMA, LAMBDA = 0.1, 0.1, 0.5
NCORES = 8
NLOC = N // NCORES  # 1024 local nodes per core
K = N // 128  # 64 global node chunks
KP = K // 2  # 32 chunk pairs
RB = NLOC // 128  # 8 local row blocks
NFP = 512  # padded feature dim

F32 = mybir.dt.float32
BF16 = mybir.dt.bfloat16
FP8 = mybir.dt.float8e4

FP8_NP = ml_dtypes.float8_e4m3
FP8_ONE = np.array([1.0], FP8_NP).view(np.uint8)[0]

# chunk-pair consumption order per input half: local-half chunks of every
# core in core order.  Pairs 4c,4c+1 = core c half 0; 4c+2,4c+3 = half 1.
KPH = [
    [4 * c + j for c in range(NCORES) for j in (0, 1)],
    [4 * c + j for c in range(NCORES) for j in (2, 3)],
]


def build_program():
    nc = bacc.Bacc(num_devices=NCORES)

    at_d = nc.dram_tensor("at_c", [128, K * NLOC], FP8, kind="ExternalInput")
    xt_d = nc.dram_tensor("xT_c", [NFP, NLOC], BF16, kind="ExternalInput")
    fcw_d = nc.dram_tensor("fc_in_w_p", [NFP, NHID], BF16, kind="ExternalInput")
    fcb_d = nc.dram_tensor("fc_in_b", [NHID], F32, kind="ExternalInput")
    c_d = nc.dram_tensor("c_vec", [NHID], F32, kind="ExternalInput")
    wg_d = nc.dram_tensor("w_gcnii", [NLAYERS, NHID, NHID], BF16, kind="ExternalInput")
    bg_d = nc.dram_tensor("b_gcnii", [NLAYERS, NHID], F32, kind="ExternalInput")
    wl_d = nc.dram_tensor("w_lin", [NLAYERS, NHID, NHID], BF16, kind="ExternalInput")
    bl_d = nc.dram_tensor("b_lin", [NLAYERS, NHID], F32, kind="ExternalInput")
    fow_d = nc.dram_tensor("fc_out_w", [NHID, NCLASS], F32, kind="ExternalInput")
    fob_d = nc.dram_tensor("fc_out_b", [NCLASS], F32, kind="ExternalInput")
    out_t = nc.dram_tensor("out_t", [NCLASS, NLOC], F32, kind="ExternalOutput")

    ident_d = nc.inline_tensor(np.eye(128, dtype=np.float32), name="ident128")

    betas = [math.log(LAMBDA / (i + 1) + 1.0) for i in range(NLAYERS)]

    with tile.TileContext(nc, num_cores=NCORES) as tc:
        with (
            tc.tile_pool(name="persist", bufs=1) as pp,
            tc.tile_pool(name="state", bufs=2) as stp,
            tc.tile_pool(name="psbp", bufs=2) as psbp,
            tc.tile_pool(name="dram", bufs=1, space="DRAM") as dram,
        ):
            # ---- persistent SBUF tiles ---------------------------------
            at_all = pp.tile([128, K * NLOC], FP8)  # 64KB/partition
            # (p, rh, kp, o, rin): o = chunk-within-pair for DoubleRow
            at_v = at_all[:].rearrange(
                "p (rh kp o rin) -> p rh kp o rin", rh=2, kp=KP, o=2
            )
            ident = pp.tile([128, 128], F32)
            ident_bf = pp.tile([128, 128], BF16)
            ones_fp8 = pp.tile([128, 256], FP8)
            nc.vector.memset(ones_fp8[:], 1.0)
            ones_dr = ones_fp8[:].rearrange("p (o m) -> p o m", o=2)
            ones_col = pp.tile([128, 1], F32)
            nc.vector.memset(ones_col[:], 1.0)
            ones128 = pp.tile([128, 128], F32)
            nc.vector.memset(ones128[:], 1.0)

            # dummy AllGather first on gpsimd: absorbs the NEFF collectives
            # init/barrier cost while the AT stream runs
            USE_DUMMY_AG = False
            if USE_DUMMY_AG:
                dummy_sb = pp.tile([128, 64], FP8)
                nc.vector.memset(dummy_sb[:], 0.0)
                dummy_in = dram.tile([128, 64], FP8, name="dummy_in")
                dummy_out = dram.tile(
                    [128 * NCORES, 64], FP8, addr_space="Shared", name="dummy_out"
                )
                nc.gpsimd.dma_start(dummy_in[:], dummy_sb[:])
                nc.gpsimd.collective_compute(
                    "AllGather",
                    mybir.AluOpType.bypass,
                    replica_groups=[list(range(NCORES))],
                    ins=[dummy_in[:].opt()],
                    outs=[dummy_out[:].opt()],
                )

            # small weight loads on gpsimd (it streams only 2MB of AT so
            # these fit before its slabs); x split across sync/scalar first
            nc.gpsimd.dma_start(ident[:], ident_d[:])
            fcw_bf = pp.tile([128, 4 * 128], BF16)
            nc.gpsimd.dma_start(
                fcw_bf[:].rearrange("p (j f) -> p j f", j=4),
                fcw_d[:].rearrange("(j p) f -> p j f", p=128),
            )
            fcb_sb = pp.tile([128, 1], F32)
            nc.gpsimd.dma_start(fcb_sb[:], fcb_d[:].rearrange("(p o) -> p o", o=1))
            c_sb = pp.tile([128, 1], F32)
            nc.gpsimd.dma_start(c_sb[:], c_d[:].rearrange("(p o) -> p o", o=1))
            nc.vector.tensor_copy(ident_bf[:], ident[:])

            with tc.tile_pool(name="fcpool", bufs=1) as fcp:
                x_sb = fcp.tile([128, 4 * NLOC], BF16)
                x_v = x_sb[:].rearrange("p (j n) -> p j n", j=4)
                xt_v = xt_d[:].rearrange("(j p) n -> p j n", p=128)
                nc.sync.dma_start(x_v[:, 0:2], xt_v[:, 0:2])
                nc.scalar.dma_start(x_v[:, 2:4], xt_v[:, 2:4])
                wg_sb = pp.tile([128, NLAYERS * 128], BF16)
                wl_bf = pp.tile([128, NLAYERS * 128], BF16)
                bg_sb = pp.tile([128, NLAYERS], F32)
                bl_sb = pp.tile([128, NLAYERS], F32)
                fow_sb = pp.tile([128, NCLASS], F32)
                fob_sb = pp.tile([NCLASS, 1], F32)
                nc.gpsimd.dma_start(
                    wg_sb[:].rearrange("p (l f) -> p l f", l=NLAYERS),
                    wg_d[:].rearrange("l p f -> p l f"),
                )
                nc.gpsimd.dma_start(
                    wl_bf[:].rearrange("p (l f) -> p l f", l=NLAYERS),
                    wl_d[:].rearrange("l p f -> p l f"),
                )
                nc.gpsimd.dma_start(bg_sb[:], bg_d[:].rearrange("l p -> p l"))
                nc.gpsimd.dma_start(bl_sb[:], bl_d[:].rearrange("l p -> p l"))
                nc.gpsimd.dma_start(fow_sb[:], fow_d[:])
                nc.gpsimd.dma_start(
                    fob_sb[:], fob_d[:].rearrange("(p o) -> p o", o=1)
                )

                # AT stream: rh0 slabs s0-s7 + rh1 s8-s11 alternate over
                # sync/scalar; gpsimd takes s12-s15 (lands before s8-s11)
                # then is free early for the cc_in DMAs
                for s in range(12):
                    sl = slice(s * 4096, (s + 1) * 4096)
                    (nc.sync if s % 2 == 0 else nc.scalar).dma_start(
                        at_all[:, sl], at_d[:, sl]
                    )
                for s in range(12, 16):
                    sl = slice(s * 4096, (s + 1) * 4096)
                    nc.gpsimd.dma_start(at_all[:, sl], at_d[:, sl])

                c01 = pp.tile([128, 1], F32)
                nc.vector.tensor_scalar_mul(c01[:], c_sb[:], GAMMA)
                fow_bf = pp.tile([128, NCLASS], BF16)
                nc.vector.tensor_copy(fow_bf[:], fow_sb[:])
                m_bf = pp.tile([128, NLAYERS * 128], BF16)
                # M'_i = (1-ALPHA) * (beta_i*wg_i + (1-beta_i)*I)
                for i in range(NLAYERS):
                    mtmp = stp.tile([128, 128], F32, tag="mtmp")
                    nc.vector.tensor_scalar_mul(
                        mtmp[:],
                        wg_sb[:, i * 128 : (i + 1) * 128],
                        (1.0 - ALPHA) * betas[i],
                    )
                    mtmp2 = stp.tile([128, 128], F32, tag="mtmp2")
                    nc.vector.tensor_scalar_mul(
                        mtmp2[:], ident[:], (1.0 - ALPHA) * (1.0 - betas[i])
                    )
                    nc.vector.tensor_add(
                        m_bf[:, i * 128 : (i + 1) * 128], mtmp[:], mtmp2[:]
                    )

                dinv_nch = pp.tile([128, RB], F32)
                b_nch = pp.tile([128, RB * 128], F32)
                b_d1 = pp.tile([128, NLOC], F32)
                b_d2 = pp.tile([128, NLOC], F32)
                h0T_01s = pp.tile([128, NLOC], BF16)
                g0_bf = pp.tile([128, NLAYERS * NLOC], BF16)
                hT = stp.tile([128, NLOC], BF16, tag="hT", name="hT_l0")
                ploc0 = pp.tile([128, RB * 128], FP8, name="ploc0")
                psb = psbp.tile([128, K * 128], FP8, tag="psb", name="psb0")
                psbv = psb[:].rearrange("p (cg hh b) -> p cg hh b", cg=NCORES, hh=2)

                with (
                    tc.tile_pool(name="ps_fc", bufs=2, space="PSUM") as psfc,
                    tc.tile_pool(name="ps_deg", bufs=1, space="PSUM") as psdeg,
                    tc.tile_pool(name="ps_p0", bufs=1, space="PSUM") as psp0,
                ):
                    deg0 = psdeg.tile([128, 512], F32, name="deg0")
                    deg1 = psdeg.tile([128, 512], F32, name="deg1")
                    sq_d = fcp.tile([128, NLOC], F32, name="sq_d")
                    ps_tr = psp0.tile([128, NLOC], F32, tag="p0big", name="ps_tr0")
                    ps_nch = psp0.tile([128, RB], F32, name="ps_nch")

                    def deg_waves(rh, pairs, start, stop):
                        # emit in expected slab-arrival order; start/stop on
                        # the first/last EMITTED matmul of the accum group
                        dst = deg0 if rh == 0 else deg1
                        for n, j in enumerate(pairs):
                            nc.tensor.matmul(
                                dst[:],
                                ones_dr,
                                at_v[:, rh, j],
                                start=(start and n == 0),
                                stop=(stop and n == len(pairs) - 1),
                                perf_mode=mybir.MatmulPerfMode.DoubleRow,
                            )

                    def dinv_half(rh):
                        # sqrt(deg+1) on the duplicated tile, then extract
                        # the node-chunk layout via PE column picks and
                        # reciprocal only on the tiny extract
                        hs = slice(rh * 512, (rh + 1) * 512)
                        nc.scalar.activation(
                            sq_d[:, hs],
                            (deg0 if rh == 0 else deg1)[:],
                            mybir.ActivationFunctionType.Sqrt,
                            bias=ones_col[:, 0:1],
                        )
                        for nb in range(rh * 4, rh * 4 + 4):
                            nc.tensor.matmul(
                                ps_nch[:, nb : nb + 1],
                                sq_d[:, nb * 128 : (nb + 1) * 128],
                                ident[:, 0:1],
                                start=True,
                                stop=True,
                                skip_group_check=True,
                            )
                        nbs = slice(rh * 4, rh * 4 + 4)
                        nc.vector.reciprocal(dinv_nch[:, nbs], ps_nch[:, nbs])

                    def ploc0_half(rh):
                        for nb in range(rh * 4, rh * 4 + 4):
                            nc.vector.tensor_scalar_mul(
                                ploc0[:, nb * 128 : (nb + 1) * 128],
                                ps_tr[:, nb * 128 : (nb + 1) * 128],
                                dinv_nch[:, nb : nb + 1],
                            )
                        cc_in = dram.tile([128, 4 * NHID], FP8, name=f"ccin0{rh}")
                        cc_out = dram.tile(
                            [128 * NCORES, 4 * NHID],
                            FP8,
                            addr_space="Shared",
                            name=f"ccout0{rh}",
                        )
                        nc.gpsimd.dma_start(
                            cc_in[:], ploc0[:, rh * 512 : (rh + 1) * 512]
                        )
                        nc.gpsimd.collective_compute(
                            "AllGather",
                            mybir.AluOpType.bypass,
                            replica_groups=[list(range(NCORES))],
                            ins=[cc_in[:].opt()],
                            outs=[cc_out[:].opt()],
                        )
                        # psb half load: 2 pieces over the 2 HWDGE queues
                        for (c0, c1), eng in zip(
                            [(0, 4), (4, 8)],
                            [nc.sync, nc.scalar],
                        ):
                            eng.dma_start(
                                psbv[:, c0:c1, rh, :],
                                cc_out[c0 * 128 : c1 * 128, :].rearrange(
                                    "(cg p) b -> p cg b", p=128
                                ),
                            )
                        return cc_out

                    # ---- phase-0 PE program ----
                    deg_waves(0, list(range(16)), start=True, stop=False)
                    # fc_in: h0^T = relu(W^T x^T + b) -> gamma blend
                    for nh in range(2):
                        ps_h = psfc.tile([128, 512], F32, tag="psfc")
                        for j in range(4):
                            nc.tensor.matmul(
                                ps_h[:],
                                fcw_bf[:, j * 128 : (j + 1) * 128],
                                x_sb[
                                    :, j * NLOC + nh * 512 : j * NLOC + (nh + 1) * 512
                                ],
                                start=(j == 0),
                                stop=(j == 3),
                            )
                        htmp = fcp.tile([128, 512], F32, tag="htmp", bufs=2)
                        nc.scalar.activation(
                            htmp[:],
                            ps_h[:],
                            mybir.ActivationFunctionType.Relu,
                            bias=fcb_sb[:, 0:1],
                        )
                        nc.scalar.activation(
                            hT[:, nh * 512 : (nh + 1) * 512],
                            htmp[:],
                            mybir.ActivationFunctionType.Identity,
                            bias=c01[:, 0:1],
                            scale=1.0 - GAMMA,
                        )
                    nc.vector.tensor_scalar_mul(
                        h0T_01s[:], hT[:], ALPHA / (1.0 - ALPHA)
                    )
                    # h0 transposes lo (PE idle while stream continues)
                    for nb in range(4):
                        nc.tensor.matmul(
                            ps_tr[:, nb * 128 : (nb + 1) * 128],
                            hT[:, nb * 128 : (nb + 1) * 128],
                            ident_bf[:],
                            start=True,
                            stop=True,
                            skip_group_check=True,
                        )
                    deg_waves(0, list(range(16, KP)), start=False, stop=True)
                    dinv_half(0)
                    cc0_h0 = ploc0_half(0)
                    # rh1: gpsimd's slabs (pairs 16-31) land before the
                    # sync/scalar tail (pairs 0-15)
                    deg_waves(
                        1,
                        list(range(16, KP)) + list(range(16)),
                        start=True,
                        stop=True,
                    )
                    for nb in range(4, 8):
                        nc.tensor.matmul(
                            ps_tr[:, nb * 128 : (nb + 1) * 128],
                            hT[:, nb * 128 : (nb + 1) * 128],
                            ident_bf[:],
                            start=True,
                            stop=True,
                            skip_group_check=True,
                        )
                    dinv_half(1)
                    cc0_h1 = ploc0_half(1)

                    # G0_i = M'_i @ (a/(1-a) h0)^T
                    for i in range(NLAYERS):
                        for nh in range(2):
                            ps_g0 = psfc.tile([128, 512], F32, tag="psfc")
                            nc.tensor.matmul(
                                ps_g0[:],
                                m_bf[:, i * 128 : (i + 1) * 128],
                                h0T_01s[:, nh * 512 : (nh + 1) * 512],
                                start=True,
                                stop=True,
                            )
                            nc.vector.tensor_copy(
                                g0_bf[
                                    :, i * NLOC + nh * 512 : i * NLOC + (nh + 1) * 512
                                ],
                                ps_g0[:],
                            )

                    # b_nch broadcast + b_d1 via PE transposes, b_d2 = b_d1^2
                    for nb in range(RB):
                        nc.vector.tensor_scalar_mul(
                            b_nch[:, nb * 128 : (nb + 1) * 128],
                            ones128[:],
                            dinv_nch[:, nb : nb + 1],
                        )
                    ps_bd = psp0.tile([128, NLOC], F32, tag="p0big", name="ps_bd")
                    for nb in range(RB):
                        nc.tensor.matmul(
                            ps_bd[:, nb * 128 : (nb + 1) * 128],
                            b_nch[:, nb * 128 : (nb + 1) * 128],
                            ident[:],
                            start=True,
                            stop=True,
                            skip_group_check=True,
                        )
                    nc.vector.tensor_copy(b_d1[:], ps_bd[:])
                    nc.vector.tensor_mul(b_d2[:], b_d1[:], b_d1[:])

            # =============== layers ===============
            with (
                tc.tile_pool(name="tmp4", bufs=4) as tp,
                tc.tile_pool(name="bfp", bufs=4) as bfp,
                tc.tile_pool(name="ps_st", bufs=1, space="PSUM") as pst,
                tc.tile_pool(name="ps_aux", bufs=2, space="PSUM") as psa,
                tc.tile_pool(name="ps_tr", bufs=2, space="PSUM") as pstr,
            ):
                ps_o = psa.tile([NCLASS, NLOC], F32, tag="auxo", name="pso", bufs=1)
                psb_cur = psb
                for i in range(NLAYERS):
                    last = i == NLAYERS - 1
                    psb_v = psb_cur[:].rearrange(
                        "p (kp o f) -> p kp o f", kp=KP, o=2
                    )
                    hT_new = stp.tile([128, NLOC], BF16, tag="hT", name=f"hT_l{i + 1}")
                    if not last:
                        ploc_next = tp.tile(
                            [128, RB * 128], FP8, tag="ploc", bufs=2,
                            name=f"ploc{i + 1}",
                        )
                        psb_next = psbp.tile(
                            [128, K * 128], FP8, tag="psb", name=f"psb{i + 1}"
                        )
                        psbv_next = psb_next[:].rearrange(
                            "p (cg hh b) -> p cg hh b", cg=NCORES, hh=2
                        )

                    st_tiles = [
                        pst.tile([128, 512], F32, tag=f"st{rh}", name=f"st{rh}_{i}")
                        for rh in range(2)
                    ]

                    def spmm(rh, hf, j0, j1, stop, psb_v=psb_v, st_tiles=st_tiles):
                        # accumulate psb chunk-pairs of input half hf into
                        # the output-half rh tile
                        for j in range(j0, j1):
                            kp = KPH[hf][j]
                            nc.tensor.matmul(
                                st_tiles[rh][:],
                                psb_v[:, kp],
                                at_v[:, rh, kp],
                                start=(hf == 0 and j == 0),
                                stop=(stop and j == j1 - 1),
                                perf_mode=mybir.MatmulPerfMode.DoubleRow,
                            )

                    def epi_front(rh, i=i, st_tiles=st_tiles, hT=hT):
                        # AH = dinv*S + dinv^2*h  (self-loop folded in, no
                        # PE identity-inject needed)
                        sl = slice(rh * 512, (rh + 1) * 512)
                        t1 = bfp.tile([128, 512], BF16, tag="t1", name=f"t1_{i}_{rh}")
                        nc.vector.tensor_mul(t1[:], st_tiles[rh][:], b_d1[:, sl])
                        t2 = bfp.tile([128, 512], BF16, tag="t2", name=f"t2_{i}_{rh}")
                        nc.vector.tensor_mul(t2[:], hT[:, sl], b_d2[:, sl])
                        ah_bf = bfp.tile(
                            [128, 512], BF16, tag="ah", name=f"ah_{i}_{rh}"
                        )
                        nc.vector.tensor_add(ah_bf[:], t1[:], t2[:])
                        return ah_bf

                    def epi_mm(rh, ah_bf, i=i):
                        sl = slice(i * NLOC + rh * 512, i * NLOC + (rh + 1) * 512)
                        ps_lin = psa.tile(
                            [128, 512], F32, tag="aux", name=f"pl_{i}_{rh}"
                        )
                        nc.tensor.matmul(
                            ps_lin[:],
                            wl_bf[:, i * 128 : (i + 1) * 128],
                            ah_bf[:],
                            start=True,
                            stop=True,
                        )
                        ps_gc = psa.tile(
                            [128, 512], F32, tag="aux", name=f"pg_{i}_{rh}"
                        )
                        # stop on the first matmul is sim-only bookkeeping;
                        # the g0 accumulate is flagged skip_group_check so
                        # the sim sees a closed group (stop is a no-op on HW)
                        nc.tensor.matmul(
                            ps_gc[:],
                            m_bf[:, i * 128 : (i + 1) * 128],
                            ah_bf[:],
                            start=True,
                            stop=True,
                        )
                        nc.tensor.matmul(
                            ps_gc[:],
                            ident_bf[:],
                            g0_bf[:, sl],
                            start=False,
                            stop=True,
                            skip_group_check=True,
                        )
                        return ps_lin, ps_gc

                    def epi_act(rh, ps_lin, ps_gc, i=i, hT_new=hT_new, last=last):
                        sl = slice(rh * 512, (rh + 1) * 512)
                        lin_sb = bfp.tile(
                            [128, 512], BF16, tag="lin", name=f"ls_{i}_{rh}"
                        )
                        nc.vector.tensor_scalar_add(
                            lin_sb[:], ps_lin[:], bl_sb[:, i : i + 1]
                        )
                        gc_sb = bfp.tile(
                            [128, 512], BF16, tag="gc", name=f"gs_{i}_{rh}"
                        )
                        nc.scalar.activation(
                            gc_sb[:],
                            ps_gc[:],
                            mybir.ActivationFunctionType.Relu,
                            bias=bg_sb[:, i : i + 1],
                        )
                        nc.vector.tensor_add(hT_new[:, sl], lin_sb[:], gc_sb[:])
                        if last:
                            nc.tensor.matmul(
                                ps_o[:, sl],
                                fow_bf[:, 0:NCLASS],
                                hT_new[:, sl],
                                start=True,
                                stop=True,
                                skip_group_check=True,
                            )

                    def epi_tr(rh, i=i, hT_new=hT_new):
                        ps_tr = pstr.tile(
                            [128, 512], F32, tag="tr", name=f"tr_{i}_{rh}"
                        )
                        for nb in range(4):
                            nc.tensor.matmul(
                                ps_tr[:, nb * 128 : (nb + 1) * 128],
                                hT_new[
                                    :, rh * 512 + nb * 128 : rh * 512 + (nb + 1) * 128
                                ],
                                ident_bf[:],
                                start=True,
                                stop=True,
                                skip_group_check=True,
                            )
                        return ps_tr

                    def epi_gather(rh, ps_tr, i=i):
                        hs = slice(rh * 512, (rh + 1) * 512)
                        nc.vector.tensor_mul(
                            ploc_next[:, hs], ps_tr[:], b_nch[:, hs]
                        )
                        cc_in = dram.tile(
                            [128, 4 * NHID], FP8, name=f"ccin{i + 1}{rh}"
                        )
                        cc_out = dram.tile(
                            [128 * NCORES, 4 * NHID],
                            FP8,
                            addr_space="Shared",
                            name=f"ccout{i + 1}{rh}",
                        )
                        nc.gpsimd.dma_start(cc_in[:], ploc_next[:, hs])
                        nc.gpsimd.collective_compute(
                            "AllGather",
                            mybir.AluOpType.bypass,
                            replica_groups=[list(range(NCORES))],
                            ins=[cc_in[:].opt()],
                            outs=[cc_out[:].opt()],
                        )
                        for (c0, c1), eng in zip(
                            [(0, 4), (4, 8)],
                            [nc.sync, nc.scalar],
                        ):
                            eng.dma_start(
                                psbv_next[:, c0:c1, rh, :],
                                cc_out[c0 * 128 : c1 * 128, :].rearrange(
                                    "(cg p) b -> p cg b", p=128
                                ),
                            )
                        return cc_out

                    # ---- layer PE program (interleaved st0/st1 per input
                    # half so only psb-half arrival gates the PE) ----
                    spmm(0, 0, 0, 16, stop=False)
                    spmm(1, 0, 0, 16, stop=False)
                    spmm(0, 1, 0, 16, stop=True)
                    ah0 = epi_front(0)
                    spmm(1, 1, 0, 8, stop=False)
                    pl0, pg0 = epi_mm(0, ah0)
                    epi_act(0, pl0, pg0)
                    spmm(1, 1, 8, 16, stop=True)
                    if not last:
                        tr0 = epi_tr(0)
                        epi_gather(0, tr0)
                    ah1 = epi_front(1)
                    pl1, pg1 = epi_mm(1, ah1)
                    epi_act(1, pl1, pg1)
                    if not last:
                        tr1 = epi_tr(1)
                        epi_gather(1, tr1)
                    hT = hT_new
                    if not last:
                        psb_cur = psb_next

                # ---- output head ----
                out_sb = tp.tile([NCLASS, NLOC], F32, tag="outsb", name="out_sb")
                nc.scalar.activation(
                    out_sb[:],
                    ps_o[:],
                    mybir.ActivationFunctionType.Identity,
                    bias=fob_sb[:, 0:1],
                )
                nc.sync.dma_start(out_t[:], out_sb[:])

    nc.compile()
    return nc


_program_cache = {}


def _get_program():
    if "nc" not in _program_cache:
        _program_cache["nc"] = build_program()
    return _program_cache["nc"]


def kernel(
    x,
    adj,
    fc_in_w,
    fc_in_b,
    c,
    w_gcnii,
    b_gcnii,
    w_lin,
    b_lin,
    fc_out_w,
    fc_out_b,
    _trace=False,
):
    x = np.asarray(x, dtype=np.float32)
    adj = np.asarray(adj, dtype=np.float32)
    x_pad = np.zeros((N, NFP), np.float32)
    x_pad[:, :NFEAT] = x
    xt = np.ascontiguousarray(x_pad.T).astype(ml_dtypes.bfloat16)  # [512, N]
    fcw_pad = np.zeros((NFP, NHID), np.float32)
    fcw_pad[:NFEAT, :] = np.asarray(fc_in_w, np.float32)
    # adj is exactly 0/1; re-encode losslessly as fp8e4 (1.0 = 0x38)
    adj8 = (adj.astype(np.uint8) * FP8_ONE).view(FP8_NP)

    shared = {
        "fc_in_w_p": fcw_pad.astype(ml_dtypes.bfloat16),
        "fc_in_b": np.asarray(fc_in_b, np.float32),
        "c_vec": np.asarray(c, np.float32),
        "w_gcnii": np.ascontiguousarray(w_gcnii).astype(ml_dtypes.bfloat16),
        "b_gcnii": np.ascontiguousarray(b_gcnii, np.float32),
        "w_lin": np.ascontiguousarray(w_lin).astype(ml_dtypes.bfloat16),
        "b_lin": np.ascontiguousarray(b_lin, np.float32),
        "fc_out_w": np.ascontiguousarray(fc_out_w, np.float32),
        "fc_out_b": np.asarray(fc_out_b, np.float32),
    }
    in_maps = []
    for cix in range(NCORES):
        r0, r1 = cix * NLOC, (cix + 1) * NLOC
        m = dict(shared)
        # r-half-major AT: [rh, rin, kc, p] -> [p, rh, kc, rin]
        slab = adj8[r0:r1, :].reshape(2, 512, K, 128)
        m["at_c"] = np.ascontiguousarray(slab.transpose(3, 0, 2, 1)).reshape(
            128, K * NLOC
        )
        m["xT_c"] = np.ascontiguousarray(xt[:, r0:r1])  # [512, NLOC] bf16
        in_maps.append(m)

    nc = _get_program()
    res = bass_utils.run_bass_kernel_spmd(
        nc, in_maps=in_maps, core_ids=list(range(NCORES)), trace=_trace
    )
    out = np.empty((N, NCLASS), np.float32)
    for cix in range(NCORES):
        out[cix * NLOC : (cix + 1) * NLOC, :] = res.results[cix]["out_t"].T
    kernel.last_exec_time_ns = res.exec_time_ns
    kernel.last_results = res
    return out


kernel.last_exec_time_ns = None
kernel.last_results = None


# revision 28
# speedup vs baseline: 1.1276x; 1.1276x over previous
"""EnhancedGCNII on 8 Trainium2 NeuronCores.

Strategy (row-sharded nodes, host-transposed fp8 adjacency), v2:
  - Core c owns node rows Rc = [c*1024, (c+1)*1024).
  - Host ships AT_c = adj[Rc, :].T as fp8e4 in r-half-major layout
    [128, 2, 64, 512]: partition p, (rh, kc, rin) with rh the local-node
    half, kc the global 128-node chunk, rin the node within the half.
    The rh-0 half streams first so its degrees (colsums) complete at
    half-stream, letting the first AllGather of dinv*h0 fire early.
  - AT streams as 16 half-slabs round-robin over the sync/scalar/gpsimd
    DMA queues; x + all weights ride the vector queue.
  - A tiny dummy AllGather issues first so the NEFF-level collectives
    barrier/init cost is absorbed during the AT stream.
  - Degrees: deg[r] = colsum of AT via all-ones stationary fp8 DoubleRow
    matmuls consuming half-slabs as they land; sqrt/reciprocal on tiny
    extracted tiles only.
  - Associativity: A_hat @ (H W') = (A_hat @ H) @ W', so per layer only
    ONE 128-feature SpMM: S^T = P'^T @ AT with P' = dinv*h gathered fp8.
    The self-loop term is folded into the epilogue without a PE matmul:
    AH = dinv*S + dinv^2*h, i.e. ah = st*b_d1 + hT*b_d2.
  - SpMM consumes gathered halves in arrival order with st0/st1 (output
    node halves) interleaved per input half, so all non-gated matmuls
    run before the second half-gather lands.
  - gcnii constant term G0_i = M'_i @ (a/(1-a) h0) precomputed; per
    layer gcnii = relu(AH@M' + G0 + bg), linear = AH@W' + b'.
  - psb (gathered stationary) loads split over 4 DMA queues; the
    tensor-queue piece is enqueued right where the PE would stall
    anyway.
  - Everything stays in transposed [feat, node] layout; biases are
    per-partition scalars. logits^T = fc_out_w^T @ h^T, host transposes.
"""

import sys
import types

sys.path.insert(0, "/opt/trn_rl_repo")

# ---------------------------------------------------------------------------
# Environment shims (axon container):
#  - antenv.axon_hooks is absent; register the NTFF profile hook ourselves so
#    trace=True yields exec_time_ns.
#  - no artifact bucket; skip uploads.
#  - walrus in this container allows only ONE semaphore wait on the CTRL
#    instruction Tile emits as the kernel-tail drain; split the waits across
#    sequential NOPs.
# ---------------------------------------------------------------------------
import antenv  # noqa: E402

if "antenv.axon_hooks" not in sys.modules:
    _mod = types.ModuleType("antenv.axon_hooks")
    _hook = [None]
    _mod.set_axon_ntff_profile_hook = lambda h: _hook.__setitem__(0, h)
    _mod.get_axon_ntff_profile_hook = lambda: _hook[0]
    sys.modules["antenv.axon_hooks"] = _mod
    antenv.axon_hooks = _mod
    try:
        from trn_agent_boot.trn_boot import _ntff_profile_via_ctypes

        _mod.set_axon_ntff_profile_hook(
            _ntff_profile_via_ctypes("/opt/axon/libaxon_pjrt.so")
        )
    except Exception as _e:
        print(f"ntff hook registration failed: {_e}", file=sys.stderr)

import numpy as np  # noqa: E402
import ml_dtypes  # noqa: E402
import concourse.bass as bass  # noqa: E402
import concourse.bacc as bacc  # noqa: E402
import concourse.mybir as mybir  # noqa: E402
import concourse.tile as tile  # noqa: E402
from concourse import bass_utils  # noqa: E402

bass_utils.upload_artifacts = lambda tmpdir: f"local://{tmpdir}"

_MAX_DRAIN_WAITS = 1


def _split_drain_and_barrier(self, tick_clock, wait_clock):
    nc = self.nc
    carrier = nc.sync.nop(hint="drain_wait_carrier", nofuse=True)
    wait_clock.add_sem_waits(
        carrier.ins, tile.ScopedClock({None: tick_clock.global_clock})
    )
    si = carrier.ins.sync_info
    if si is not None and len(si.on_wait) > _MAX_DRAIN_WAITS:
        waits = list(si.on_wait)
        carrier.ins.sync_info = mybir.SyncInfo(
            on_wait=waits[:_MAX_DRAIN_WAITS], on_update=list(si.on_update)
        )
        for i in range(_MAX_DRAIN_WAITS, len(waits), _MAX_DRAIN_WAITS):
            extra = nc.sync.nop(hint="drain_wait_split", nofuse=True)
            extra.ins.sync_info = mybir.SyncInfo(
                on_wait=waits[i : i + _MAX_DRAIN_WAITS], on_update=[]
            )
    nc.sync.drain()
    nc.all_engine_barrier()
    assert self.sems is not None
    popped = nc._tile_sem_poison_stack.pop()
    assert popped is self._sem_poison
    nc.clear_and_free_semaphores(list(self.sems.allocated().values()))
    nc.all_engine_barrier()


tile.TileContext._drain_and_barrier = _split_drain_and_barrier

# ---------------------------------------------------------------------------
# Problem constants (hardcoded per the harness contract)
# ---------------------------------------------------------------------------
import math  # noqa: E402

N, NFEAT, NHID, NCLASS, NLAYERS = 8192, 500, 128, 40, 4
ALPHA, GAMMA, LAMBDA = 0.1, 0.1, 0.5
NCORES = 8
NLOC = N // NCORES  # 1024 local nodes per core
K = N // 128  # 64 global node chunks
KP = K // 2  # 32 chunk pairs
RB = NLOC // 128  # 8 local row blocks
NFP = 512  # padded feature dim

F32 = mybir.dt.float32
BF16 = mybir.dt.bfloat16
FP8 = mybir.dt.float8e4

FP8_NP = ml_dtypes.float8_e4m3
FP8_ONE = np.array([1.0], FP8_NP).view(np.uint8)[0]

# chunk-pair consumption order per input half: local-half chunks of every
# core in core order.  Pairs 4c,4c+1 = core c half 0; 4c+2,4c+3 = half 1.
KPH = [
    [4 * c + j for c in range(NCORES) for j in (0, 1)],
    [4 * c + j for c in range(NCORES) for j in (2, 3)],
]


def build_program():
    nc = bacc.Bacc(num_devices=NCORES)

    at_d = nc.dram_tensor("at_c", [128, K * NLOC], FP8, kind="ExternalInput")
    xt_d = nc.dram_tensor("xT_c", [NFP, NLOC], BF16, kind="ExternalInput")
    fcw_d = nc.dram_tensor("fc_in_w_p", [NFP, NHID], BF16, kind="ExternalInput")
    fcb_d = nc.dram_tensor("fc_in_b", [NHID], F32, kind="ExternalInput")
    c_d = nc.dram_tensor("c_vec", [NHID], F32, kind="ExternalInput")
    wg_d = nc.dram_tensor("w_gcnii", [NLAYERS, NHID, NHID], BF16, kind="ExternalInput")
    bg_d = nc.dram_tensor("b_gcnii", [NLAYERS, NHID], F32, kind="ExternalInput")
    wl_d = nc.dram_tensor("w_lin", [NLAYERS, NHID, NHID], BF16, kind="ExternalInput")
    bl_d = nc.dram_tensor("b_lin", [NLAYERS, NHID], F32, kind="ExternalInput")
    fow_d = nc.dram_tensor("fc_out_w", [NHID, NCLASS], F32, kind="ExternalInput")
    fob_d = nc.dram_tensor("fc_out_b", [NCLASS], F32, kind="ExternalInput")
    out_t = nc.dram_tensor("out_t", [NCLASS, NLOC], F32, kind="ExternalOutput")

    ident_d = nc.inline_tensor(np.eye(128, dtype=np.float32), name="ident128")

    betas = [math.log(LAMBDA / (i + 1) + 1.0) for i in range(NLAYERS)]

    with tile.TileContext(nc, num_cores=NCORES) as tc:
        with (
            tc.tile_pool(name="persist", bufs=1) as pp,
            tc.tile_pool(name="state", bufs=2) as stp,
            tc.tile_pool(name="psbp", bufs=2) as psbp,
            tc.tile_pool(name="dram", bufs=1, space="DRAM") as dram,
        ):
            # ---- persistent SBUF tiles ---------------------------------
            at_all = pp.tile([128, K * NLOC], FP8)  # 64KB/partition
            # (p, rh, kp, o, rin): o = chunk-within-pair for DoubleRow
            at_v = at_all[:].rearrange(
                "p (rh kp o rin) -> p rh kp o rin", rh=2, kp=KP, o=2
            )
            ident = pp.tile([128, 128], F32)
            ident_bf = pp.tile([128, 128], BF16)
            ones_fp8 = pp.tile([128, 256], FP8)
            nc.vector.memset(ones_fp8[:], 1.0)
            ones_dr = ones_fp8[:].rearrange("p (o m) -> p o m", o=2)
            ones_col = pp.tile([128, 1], F32)
            nc.vector.memset(ones_col[:], 1.0)
            ones128 = pp.tile([128, 128], F32)
            nc.vector.memset(ones128[:], 1.0)

            # dummy AllGather FIRST on gpsimd: arms the CC stream / absorbs
            # the NEFF collectives rendezvous while the AT stream runs.
            # gpsimd carries ONLY latency-critical DMAs (SWDGE desc-gen
            # costs engine time and its queue must be empty at cc_in time).
            dummy_sb = pp.tile([128, 64], FP8)
            nc.vector.memset(dummy_sb[:], 0.0)
            dummy_in = dram.tile([128, 64], FP8, name="dummy_in")
            dummy_out = dram.tile(
                [128 * NCORES, 64], FP8, addr_space="Shared", name="dummy_out"
            )
            nc.gpsimd.dma_start(dummy_in[:], dummy_sb[:])
            nc.gpsimd.collective_compute(
                "AllGather",
                mybir.AluOpType.bypass,
                replica_groups=[list(range(NCORES))],
                ins=[dummy_in[:].opt()],
                outs=[dummy_out[:].opt()],
            )

            # small loads at the head of the sync/scalar queues
            nc.sync.dma_start(ident[:], ident_d[:])
            fcb_sb = pp.tile([128, 1], F32)
            nc.sync.dma_start(fcb_sb[:], fcb_d[:].rearrange("(p o) -> p o", o=1))
            c_sb = pp.tile([128, 1], F32)
            nc.sync.dma_start(c_sb[:], c_d[:].rearrange("(p o) -> p o", o=1))
            fcw_bf = pp.tile([128, 4 * 128], BF16)
            nc.scalar.dma_start(
                fcw_bf[:].rearrange("p (j f) -> p j f", j=4),
                fcw_d[:].rearrange("(j p) f -> p j f", p=128),
            )
            nc.vector.tensor_copy(ident_bf[:], ident[:])

            with tc.tile_pool(name="fcpool", bufs=1) as fcp:
                wg_sb = pp.tile([128, NLAYERS * 128], BF16)
                wl_bf = pp.tile([128, NLAYERS * 128], BF16)
                bg_sb = pp.tile([128, NLAYERS], F32)
                bl_sb = pp.tile([128, NLAYERS], F32)
                fow_sb = pp.tile([128, NCLASS], F32)
                fob_sb = pp.tile([NCLASS, 1], F32)
                nc.scalar.dma_start(
                    wg_sb[:].rearrange("p (l f) -> p l f", l=NLAYERS),
                    wg_d[:].rearrange("l p f -> p l f"),
                )
                nc.scalar.dma_start(
                    wl_bf[:].rearrange("p (l f) -> p l f", l=NLAYERS),
                    wl_d[:].rearrange("l p f -> p l f"),
                )
                nc.scalar.dma_start(bg_sb[:], bg_d[:].rearrange("l p -> p l"))
                nc.scalar.dma_start(bl_sb[:], bl_d[:].rearrange("l p -> p l"))
                nc.scalar.dma_start(fow_sb[:], fow_d[:])
                nc.scalar.dma_start(
                    fob_sb[:], fob_d[:].rearrange("(p o) -> p o", o=1)
                )
                x_sb = fcp.tile([128, 4 * NLOC], BF16)
                x_v = x_sb[:].rearrange("p (j n) -> p j n", j=4)
                xt_v = xt_d[:].rearrange("(j p) n -> p j n", p=128)
                nc.sync.dma_start(x_v[:, 0:2], xt_v[:, 0:2])
                nc.scalar.dma_start(x_v[:, 2:4], xt_v[:, 2:4])

                # AT stream: all 16 half-slabs on sync/scalar (rh0 first)
                for s in range(16):
                    sl = slice(s * 4096, (s + 1) * 4096)
                    (nc.sync if s % 2 == 0 else nc.scalar).dma_start(
                        at_all[:, sl], at_d[:, sl]
                    )

                c01 = pp.tile([128, 1], F32)
                nc.vector.tensor_scalar_mul(c01[:], c_sb[:], GAMMA)
                fow_bf = pp.tile([128, NCLASS], BF16)
                nc.vector.tensor_copy(fow_bf[:], fow_sb[:])
                m_bf = pp.tile([128, NLAYERS * 128], BF16)
                # M'_i = (1-ALPHA) * (beta_i*wg_i + (1-beta_i)*I)
                for i in range(NLAYERS):
                    mtmp = stp.tile([128, 128], F32, tag="mtmp")
                    nc.vector.tensor_scalar_mul(
                        mtmp[:],
                        wg_sb[:, i * 128 : (i + 1) * 128],
                        (1.0 - ALPHA) * betas[i],
                    )
                    mtmp2 = stp.tile([128, 128], F32, tag="mtmp2")
                    nc.vector.tensor_scalar_mul(
                        mtmp2[:], ident[:], (1.0 - ALPHA) * (1.0 - betas[i])
                    )
                    nc.vector.tensor_add(
                        m_bf[:, i * 128 : (i + 1) * 128], mtmp[:], mtmp2[:]
                    )

                dinv_nch = pp.tile([128, RB], F32)
                b_nch = pp.tile([128, RB * 128], F32)
                b_d1 = pp.tile([128, NLOC], F32)
                b_d2 = pp.tile([128, NLOC], F32)
                h0T_01s = pp.tile([128, NLOC], BF16)
                g0_bf = pp.tile([128, NLAYERS * NLOC], BF16)
                hT = stp.tile([128, NLOC], BF16, tag="hT", name="hT_l0")
                ploc0 = pp.tile([128, RB * 128], FP8, name="ploc0")
                psb = psbp.tile([128, K * 128], FP8, tag="psb", name="psb0")
                psbv = psb[:].rearrange("p (cg hh b) -> p cg hh b", cg=NCORES, hh=2)

                with (
                    tc.tile_pool(name="ps_fc", bufs=2, space="PSUM") as psfc,
                    tc.tile_pool(name="ps_deg", bufs=1, space="PSUM") as psdeg,
                    tc.tile_pool(name="ps_p0", bufs=1, space="PSUM") as psp0,
                ):
                    deg0 = psdeg.tile([128, 512], F32, name="deg0")
                    deg1 = psdeg.tile([128, 512], F32, name="deg1")
                    sq_d = fcp.tile([128, NLOC], F32, name="sq_d")
                    ps_tr = psp0.tile([128, NLOC], F32, tag="p0big", name="ps_tr0")
                    ps_nch = psp0.tile([128, RB], F32, name="ps_nch")

                    def deg_waves(rh, pairs, start, stop):
                        # emit in expected slab-arrival order; start/stop on
                        # the first/last EMITTED matmul of the accum group
                        dst = deg0 if rh == 0 else deg1
                        for n, j in enumerate(pairs):
                            nc.tensor.matmul(
                                dst[:],
                                ones_dr,
                                at_v[:, rh, j],
                                start=(start and n == 0),
                                stop=(stop and n == len(pairs) - 1),
                                perf_mode=mybir.MatmulPerfMode.DoubleRow,
                            )

                    def dinv_half(rh):
                        # sqrt(deg+1) on the duplicated tile, then extract
                        # the node-chunk layout via PE column picks and
                        # reciprocal only on the tiny extract
                        hs = slice(rh * 512, (rh + 1) * 512)
                        nc.scalar.activation(
                            sq_d[:, hs],
                            (deg0 if rh == 0 else deg1)[:],
                            mybir.ActivationFunctionType.Sqrt,
                            bias=ones_col[:, 0:1],
                        )
                        for nb in range(rh * 4, rh * 4 + 4):
                            nc.tensor.matmul(
                                ps_nch[:, nb : nb + 1],
                                sq_d[:, nb * 128 : (nb + 1) * 128],
                                ident[:, 0:1],
                                start=True,
                                stop=True,
                                skip_group_check=True,
                            )
                        nbs = slice(rh * 4, rh * 4 + 4)
                        nc.vector.reciprocal(dinv_nch[:, nbs], ps_nch[:, nbs])

                    def ploc0_half(rh):
                        for nb in range(rh * 4, rh * 4 + 4):
                            nc.vector.tensor_scalar_mul(
                                ploc0[:, nb * 128 : (nb + 1) * 128],
                                ps_tr[:, nb * 128 : (nb + 1) * 128],
                                dinv_nch[:, nb : nb + 1],
                            )
                        cc_in = dram.tile([128, 4 * NHID], FP8, name=f"ccin0{rh}")
                        cc_out = dram.tile(
                            [128 * NCORES, 4 * NHID],
                            FP8,
                            addr_space="Shared",
                            name=f"ccout0{rh}",
                        )
                        nc.gpsimd.dma_start(
                            cc_in[:], ploc0[:, rh * 512 : (rh + 1) * 512]
                        )
                        nc.gpsimd.collective_compute(
                            "AllGather",
                            mybir.AluOpType.bypass,
                            replica_groups=[list(range(NCORES))],
                            ins=[cc_in[:].opt()],
                            outs=[cc_out[:].opt()],
                        )
                        return cc_out

                    def psb0_pieces(rh, cc_out):
                        # 3-way piece load, emitted only after the phase-0
                        # ACT compute so the scalar ENGINE never blocks on
                        # the AG-done wait ahead of sqrt/relu work
                        for (c0, c1), eng in zip(
                            [(0, 3), (3, 6), (6, 8)],
                            [nc.sync, nc.scalar, nc.gpsimd],
                        ):
                            eng.dma_start(
                                psbv[:, c0:c1, rh, :],
                                cc_out[c0 * 128 : c1 * 128, :].rearrange(
                                    "(cg p) b -> p cg b", p=128
                                ),
                            )

                    # ---- phase-0 PE program ----
                    deg_waves(0, list(range(16)), start=True, stop=False)
                    # fc_in: h0^T = relu(W^T x^T + b) -> gamma blend
                    for nh in range(2):
                        ps_h = psfc.tile([128, 512], F32, tag="psfc")
                        for j in range(4):
                            nc.tensor.matmul(
                                ps_h[:],
                                fcw_bf[:, j * 128 : (j + 1) * 128],
                                x_sb[
                                    :, j * NLOC + nh * 512 : j * NLOC + (nh + 1) * 512
                                ],
                                start=(j == 0),
                                stop=(j == 3),
                            )
                        htmp = fcp.tile([128, 512], F32, tag="htmp", bufs=2)
                        nc.scalar.activation(
                            htmp[:],
                            ps_h[:],
                            mybir.ActivationFunctionType.Relu,
                            bias=fcb_sb[:, 0:1],
                        )
                        nc.scalar.activation(
                            hT[:, nh * 512 : (nh + 1) * 512],
                            htmp[:],
                            mybir.ActivationFunctionType.Identity,
                            bias=c01[:, 0:1],
                            scale=1.0 - GAMMA,
                        )
                    nc.vector.tensor_scalar_mul(
                        h0T_01s[:], hT[:], ALPHA / (1.0 - ALPHA)
                    )
                    # h0 transposes lo (PE idle while stream continues)
                    for nb in range(4):
                        nc.tensor.matmul(
                            ps_tr[:, nb * 128 : (nb + 1) * 128],
                            hT[:, nb * 128 : (nb + 1) * 128],
                            ident_bf[:],
                            start=True,
                            stop=True,
                            skip_group_check=True,
                        )
                    deg_waves(0, list(range(16, KP)), start=False, stop=True)
                    dinv_half(0)
                    cc0_h0 = ploc0_half(0)
                    # rh1: gpsimd's slabs (pairs 16-31) land before the
                    # sync/scalar tail (pairs 0-15)
                    deg_waves(
                        1,
                        list(range(16, KP)) + list(range(16)),
                        start=True,
                        stop=True,
                    )
                    for nb in range(4, 8):
                        nc.tensor.matmul(
                            ps_tr[:, nb * 128 : (nb + 1) * 128],
                            hT[:, nb * 128 : (nb + 1) * 128],
                            ident_bf[:],
                            start=True,
                            stop=True,
                            skip_group_check=True,
                        )
                    dinv_half(1)
                    cc0_h1 = ploc0_half(1)
                    psb0_pieces(0, cc0_h0)
                    psb0_pieces(1, cc0_h1)

                    # G0_i = M'_i @ (a/(1-a) h0)^T
                    for i in range(NLAYERS):
                        for nh in range(2):
                            ps_g0 = psfc.tile([128, 512], F32, tag="psfc")
                            nc.tensor.matmul(
                                ps_g0[:],
                                m_bf[:, i * 128 : (i + 1) * 128],
                                h0T_01s[:, nh * 512 : (nh + 1) * 512],
                                start=True,
                                stop=True,
                            )
                            nc.vector.tensor_copy(
                                g0_bf[
                                    :, i * NLOC + nh * 512 : i * NLOC + (nh + 1) * 512
                                ],
                                ps_g0[:],
                            )

                    # b_nch broadcast + b_d1 via PE transposes, b_d2 = b_d1^2
                    for nb in range(RB):
                        nc.vector.tensor_scalar_mul(
                            b_nch[:, nb * 128 : (nb + 1) * 128],
                            ones128[:],
                            dinv_nch[:, nb : nb + 1],
                        )
                    ps_bd = psp0.tile([128, NLOC], F32, tag="p0big", name="ps_bd")
                    for nb in range(RB):
                        nc.tensor.matmul(
                            ps_bd[:, nb * 128 : (nb + 1) * 128],
                            b_nch[:, nb * 128 : (nb + 1) * 128],
                            ident[:],
                            start=True,
                            stop=True,
                            skip_group_check=True,
                        )
                    nc.vector.tensor_copy(b_d1[:], ps_bd[:])
                    nc.vector.tensor_mul(b_d2[:], b_d1[:], b_d1[:])

            # =============== layers ===============
            with (
                tc.tile_pool(name="tmp4", bufs=4) as tp,
                tc.tile_pool(name="bfp", bufs=4) as bfp,
                tc.tile_pool(name="ps_st", bufs=1, space="PSUM") as pst,
                tc.tile_pool(name="ps_aux", bufs=2, space="PSUM") as psa,
                tc.tile_pool(name="ps_tr", bufs=2, space="PSUM") as pstr,
            ):
                ps_o = psa.tile([NCLASS, NLOC], F32, tag="auxo", name="pso", bufs=1)
                psb_cur = psb
                for i in range(NLAYERS):
                    last = i == NLAYERS - 1
                    psb_v = psb_cur[:].rearrange(
                        "p (kp o f) -> p kp o f", kp=KP, o=2
                    )
                    hT_new = stp.tile([128, NLOC], BF16, tag="hT", name=f"hT_l{i + 1}")
                    if not last:
                        ploc_next = tp.tile(
                            [128, RB * 128], FP8, tag="ploc", bufs=2,
                            name=f"ploc{i + 1}",
                        )
                        psb_next = psbp.tile(
                            [128, K * 128], FP8, tag="psb", name=f"psb{i + 1}"
                        )
                        psbv_next = psb_next[:].rearrange(
                            "p (cg hh b) -> p cg hh b", cg=NCORES, hh=2
                        )

                    st_tiles = [
                        pst.tile([128, 512], F32, tag=f"st{rh}", name=f"st{rh}_{i}")
                        for rh in range(2)
                    ]

                    def spmm(rh, hf, j0, j1, stop, psb_v=psb_v, st_tiles=st_tiles):
                        # accumulate psb chunk-pairs of input half hf into
                        # the output-half rh tile
                        for j in range(j0, j1):
                            kp = KPH[hf][j]
                            nc.tensor.matmul(
                                st_tiles[rh][:],
                                psb_v[:, kp],
                                at_v[:, rh, kp],
                                start=(hf == 0 and j == 0),
                                stop=(stop and j == j1 - 1),
                                perf_mode=mybir.MatmulPerfMode.DoubleRow,
                            )

                    def epi_front(rh, i=i, st_tiles=st_tiles, hT=hT):
                        # AH = dinv*S + dinv^2*h  (self-loop folded in, no
                        # PE identity-inject needed)
                        sl = slice(rh * 512, (rh + 1) * 512)
                        t1 = bfp.tile([128, 512], BF16, tag="t1", name=f"t1_{i}_{rh}")
                        nc.vector.tensor_mul(t1[:], st_tiles[rh][:], b_d1[:, sl])
                        t2 = bfp.tile([128, 512], BF16, tag="t2", name=f"t2_{i}_{rh}")
                        nc.vector.tensor_mul(t2[:], hT[:, sl], b_d2[:, sl])
                        ah_bf = bfp.tile(
                            [128, 512], BF16, tag="ah", name=f"ah_{i}_{rh}"
                        )
                        nc.vector.tensor_add(ah_bf[:], t1[:], t2[:])
                        return ah_bf

                    def epi_mm(rh, ah_bf, i=i):
                        sl = slice(i * NLOC + rh * 512, i * NLOC + (rh + 1) * 512)
                        ps_lin = psa.tile(
                            [128, 512], F32, tag="aux", name=f"pl_{i}_{rh}"
                        )
                        nc.tensor.matmul(
                            ps_lin[:],
                            wl_bf[:, i * 128 : (i + 1) * 128],
                            ah_bf[:],
                            start=True,
                            stop=True,
                        )
                        ps_gc = psa.tile(
                            [128, 512], F32, tag="aux", name=f"pg_{i}_{rh}"
                        )
                        # stop on the first matmul is sim-only bookkeeping;
                        # the g0 accumulate is flagged skip_group_check so
                        # the sim sees a closed group (stop is a no-op on HW)
                        nc.tensor.matmul(
                            ps_gc[:],
                            m_bf[:, i * 128 : (i + 1) * 128],
                            ah_bf[:],
                            start=True,
                            stop=True,
                        )
                        nc.tensor.matmul(
                            ps_gc[:],
                            ident_bf[:],
                            g0_bf[:, sl],
                            start=False,
                            stop=True,
                            skip_group_check=True,
                        )
                        return ps_lin, ps_gc

                    def epi_act(rh, ps_lin, ps_gc, i=i, hT_new=hT_new, last=last):
                        sl = slice(rh * 512, (rh + 1) * 512)
                        lin_sb = bfp.tile(
                            [128, 512], BF16, tag="lin", name=f"ls_{i}_{rh}"
                        )
                        nc.vector.tensor_scalar_add(
                            lin_sb[:], ps_lin[:], bl_sb[:, i : i + 1]
                        )
                        gc_sb = bfp.tile(
                            [128, 512], BF16, tag="gc", name=f"gs_{i}_{rh}"
                        )
                        nc.scalar.activation(
                            gc_sb[:],
                            ps_gc[:],
                            mybir.ActivationFunctionType.Relu,
                            bias=bg_sb[:, i : i + 1],
                        )
                        nc.vector.tensor_add(hT_new[:, sl], lin_sb[:], gc_sb[:])
                        if last:
                            nc.tensor.matmul(
                                ps_o[:, sl],
                                fow_bf[:, 0:NCLASS],
                                hT_new[:, sl],
                                start=True,
                                stop=True,
                                skip_group_check=True,
                            )

                    def epi_tr(rh, i=i, hT_new=hT_new):
                        ps_tr = pstr.tile(
                            [128, 512], F32, tag="tr", name=f"tr_{i}_{rh}"
                        )
                        for nb in range(4):
                            nc.tensor.matmul(
                                ps_tr[:, nb * 128 : (nb + 1) * 128],
                                hT_new[
                                    :, rh * 512 + nb * 128 : rh * 512 + (nb + 1) * 128
                                ],
                                ident_bf[:],
                                start=True,
                                stop=True,
                                skip_group_check=True,
                            )
                        return ps_tr

                    def epi_gather(rh, ps_tr, i=i):
                        hs = slice(rh * 512, (rh + 1) * 512)
                        nc.vector.tensor_mul(
                            ploc_next[:, hs], ps_tr[:], b_nch[:, hs]
                        )
                        cc_in = dram.tile(
                            [128, 4 * NHID], FP8, name=f"ccin{i + 1}{rh}"
                        )
                        cc_out = dram.tile(
                            [128 * NCORES, 4 * NHID],
                            FP8,
                            addr_space="Shared",
                            name=f"ccout{i + 1}{rh}",
                        )
                        # cc_in split sync+gpsimd for latency (scalar must
                        # stay free for ACT compute; sync has no compute)
                        nc.sync.dma_start(
                            cc_in[:, 0:256],
                            ploc_next[:, rh * 512 : rh * 512 + 256],
                        )
                        nc.gpsimd.dma_start(
                            cc_in[:, 256:512],
                            ploc_next[:, rh * 512 + 256 : rh * 512 + 512],
                        )
                        nc.gpsimd.collective_compute(
                            "AllGather",
                            mybir.AluOpType.bypass,
                            replica_groups=[list(range(NCORES))],
                            ins=[cc_in[:].opt()],
                            outs=[cc_out[:].opt()],
                        )
                        return cc_out

                    def psb_pieces(rh, cc_out):
                        for (c0, c1), eng in zip(
                            [(0, 3), (3, 6), (6, 8)],
                            [nc.sync, nc.scalar, nc.gpsimd],
                        ):
                            eng.dma_start(
                                psbv_next[:, c0:c1, rh, :],
                                cc_out[c0 * 128 : c1 * 128, :].rearrange(
                                    "(cg p) b -> p cg b", p=128
                                ),
                            )

                    # ---- layer PE program (interleaved st0/st1 per input
                    # half so only psb-half arrival gates the PE) ----
                    spmm(0, 0, 0, 16, stop=False)
                    spmm(1, 0, 0, 16, stop=False)
                    spmm(0, 1, 0, 16, stop=True)
                    ah0 = epi_front(0)
                    spmm(1, 1, 0, 8, stop=False)
                    pl0, pg0 = epi_mm(0, ah0)
                    epi_act(0, pl0, pg0)
                    spmm(1, 1, 8, 16, stop=True)
                    if not last:
                        tr0 = epi_tr(0)
                        cc_h0 = epi_gather(0, tr0)
                    ah1 = epi_front(1)
                    pl1, pg1 = epi_mm(1, ah1)
                    epi_act(1, pl1, pg1)
                    if not last:
                        tr1 = epi_tr(1)
                        cc_h1 = epi_gather(1, tr1)
                        # pieces emitted after all this layer's ACT compute
                        psb_pieces(0, cc_h0)
                        psb_pieces(1, cc_h1)
                    hT = hT_new
                    if not last:
                        psb_cur = psb_next

                # ---- output head ----
                out_sb = tp.tile([NCLASS, NLOC], F32, tag="outsb", name="out_sb")
                nc.scalar.activation(
                    out_sb[:],
                    ps_o[:],
                    mybir.ActivationFunctionType.Identity,
                    bias=fob_sb[:, 0:1],
                )
                nc.sync.dma_start(out_t[:], out_sb[:])

    nc.compile()
    return nc


_program_cache = {}


def _get_program():
    if "nc" not in _program_cache:
        _program_cache["nc"] = build_program()
    return _program_cache["nc"]


def kernel(
    x,
    adj,
    fc_in_w,
    fc_in_b,
    c,
    w_gcnii,
    b_gcnii,
    w_lin,
    b_lin,
    fc_out_w,
    fc_out_b,
    _trace=False,
):
    x = np.asarray(x, dtype=np.float32)
    adj = np.asarray(adj, dtype=np.float32)
    x_pad = np.zeros((N, NFP), np.float32)
    x_pad[:, :NFEAT] = x
    xt = np.ascontiguousarray(x_pad.T).astype(ml_dtypes.bfloat16)  # [512, N]
    fcw_pad = np.zeros((NFP, NHID), np.float32)
    fcw_pad[:NFEAT, :] = np.asarray(fc_in_w, np.float32)
    # adj is exactly 0/1; re-encode losslessly as fp8e4 (1.0 = 0x38)
    adj8 = (adj.astype(np.uint8) * FP8_ONE).view(FP8_NP)

    shared = {
        "fc_in_w_p": fcw_pad.astype(ml_dtypes.bfloat16),
        "fc_in_b": np.asarray(fc_in_b, np.float32),
        "c_vec": np.asarray(c, np.float32),
        "w_gcnii": np.ascontiguousarray(w_gcnii).astype(ml_dtypes.bfloat16),
        "b_gcnii": np.ascontiguousarray(b_gcnii, np.float32),
        "w_lin": np.ascontiguousarray(w_lin).astype(ml_dtypes.bfloat16),
        "b_lin": np.ascontiguousarray(b_lin, np.float32),
        "fc_out_w": np.ascontiguousarray(fc_out_w, np.float32),
        "fc_out_b": np.asarray(fc_out_b, np.float32),
    }
    in_maps = []
    for cix in range(NCORES):
        r0, r1 = cix * NLOC, (cix + 1) * NLOC
        m = dict(shared)
        # r-half-major AT: [rh, rin, kc, p] -> [p, rh, kc, rin]
        slab = adj8[r0:r1, :].reshape(2, 512, K, 128)
        m["at_c"] = np.ascontiguousarray(slab.transpose(3, 0, 2, 1)).reshape(
            128, K * NLOC
        )
        m["xT_c"] = np.ascontiguousarray(xt[:, r0:r1])  # [512, NLOC] bf16
        in_maps.append(m)

    nc = _get_program()
    res = bass_utils.run_bass_kernel_spmd(
        nc, in_maps=in_maps, core_ids=list(range(NCORES)), trace=_trace
    )
    out = np.empty((N, NCLASS), np.float32)
    for cix in range(NCORES):
        out[cix * NLOC : (cix + 1) * NLOC, :] = res.results[cix]["out_t"].T
    kernel.last_exec_time_ns = res.exec_time_ns
    kernel.last_results = res
    return out


kernel.last_exec_time_ns = None
kernel.last_results = None
